# revision 7
# baseline (speedup 1.0000x reference)
"""BiLSTM kernel for 8 Trainium2 NeuronCores — latency-optimized scan.

Design (vs the fp32 per-direction baseline):
- Both LSTM directions stacked on the partition axis (fwd chain on partitions
  0:64, bwd on 64:128), bc=32 batch columns per core; every elementwise
  instruction covers both chains at once.
- h-projections: ONE bf16 matmul per gate with a block-diagonal
  [U_f 0; 0 U_b] lhsT (k=128, m=128, 1 cycle/row vs fp32's 4).
- Gate columns [f|i|g2|o] with g preactivations doubled on host: a single
  sigmoid ACT covers f,i,g2 (tanh(g) = 2*sigmoid(2g)-1 recovered by one DVE
  tensor_scalar), sigma(o) runs in the ACT gap before tanh(c'). This keeps
  the ACT engine (185ns SBUF access latency per op) off the critical path
  as much as possible.
- e-projections (bf16) run pipelined PF steps ahead; gate PSUM banks are
  armed per partition-half (pending-zero is per-partition-range).
- Embedding gather on-device: indirect row DMA (Pool SWDGE) + PE transpose +
  cast-copy to a resident bf16 eT tile (copies alternate DVE/ACT; GPSIMD
  cannot read PSUM). Gather work is clock-paced via tile_wait_until so the
  Tile scheduler cannot drain it ahead of the scan and flood the step
  engines. The host folds a ones-column into emb so the bias row of eT
  comes out of the transpose for free.
- c and all activations stay fp32; only matmul operands (h, U, W, eT) are
  bf16 (measured end-to-end rel err ~7e-4 vs the 2e-2 gate).
"""

import sys

sys.path.insert(0, "/opt/trn_rl_repo")

import numpy as np
import ml_dtypes

import concourse.bacc as bacc
import concourse.bass as bass
import concourse.mybir as mybir
import concourse.tile as tile
from concourse.bass_utils import run_bass_kernel_spmd
from concourse.masks import make_identity

F32 = mybir.dt.float32
BF16 = mybir.dt.bfloat16
NP_BF16 = ml_dtypes.bfloat16
AF = mybir.ActivationFunctionType
ALU = mybir.AluOpType

V, E, HID, B, S = 50000, 100, 64, 256, 512
N_CORES = 8
BC = B // N_CORES  # 32 batch rows per core; both directions stacked on partitions
K = E + 1  # contraction dim: embedding dims + ones row (bias)

_built = {}
_GATHER_CFG = [8, 3]
# Gather pacing: chunk c's trigger/transform are given a scheduler
# wait-until timestamp anchored at the step that consumes it, so the Tile
# scheduler cannot drain the whole gather ahead of the scan (which floods
# the ACT/DVE/PE queues and slows every step until the gather finishes).
_STEP_EST_NS = [1900]
_PRO_EST_NS = [10000]
_TRIG_OFF_NS = [0]
_XF_OFF_NS = [1500]
_COPY_MOD = [2]


def _build(s_len=S, bc=BC, repeats=1, gather=True, scheme="o6", gather_reps=False):
    """Build + compile the SPMD program (both-direction LSTM scan).

    scheme: ACT-instruction packing for the gate nonlinearities.
      o1: sigmoid(f,i,o) then tanh(g)
      o3: sigmoid(f,i), tanh(g), sigmoid(o)
      o5: single sigmoid(f,i,o,g2) (g preacts doubled on host);
          tg = 2*sg2 - 1 via DVE tensor_scalar
    repeats > 1 reruns the scan (state reset in between; gather only on the
    first) to measure pure scan time as a slope."""
    key = (s_len, bc, repeats, gather, scheme, gather_reps, tuple(_GATHER_CFG),
           _STEP_EST_NS[0], _PRO_EST_NS[0], _TRIG_OFF_NS[0], _XF_OFF_NS[0],
           _COPY_MOD[0])
    if key in _built:
        return _built[key]

    nc = bacc.Bacc("TRN2", target_bir_lowering=False, debug=False, num_devices=N_CORES,
                   dynamic_dma_scratch_size=65536)

    n_tok2 = s_len * 2 * bc  # both directions
    n_chunks = (n_tok2 + 127) // 128
    if gather:
        # emb is host-augmented with a 101st column of ones so the gather +
        # transpose deliver the bias row of eT for free; bf16 halves the
        # gather DMA, transpose, and copy cost (e-proj is bf16 anyway)
        emb_d = nc.dram_tensor("emb", [V, K], BF16, kind="ExternalInput")
        idx_d = nc.dram_tensor("idx", [128, n_chunks], mybir.dt.int32,
                               kind="ExternalInput")
    else:
        eT_d = nc.dram_tensor("eT", [K, n_tok2], BF16, kind="ExternalInput")
    w_d = nc.dram_tensor("w_all", [K, 512], BF16, kind="ExternalInput")
    u_d = nc.dram_tensor("ubd", [128, 512], BF16, kind="ExternalInput")
    y = nc.dram_tensor("y", [128, 2 * bc], F32, kind="ExternalOutput")

    PF = 3  # e-projection lookahead (steps)
    PFCH = _GATHER_CFG[0]  # gather trigger prefetch (chunks)
    CPF = _GATHER_CFG[1]  # transform (transpose+copy) lookahead (chunks)

    with tile.TileContext(nc) as tc:
        with (
            tc.tile_pool(name="const", bufs=1) as cpool,
            tc.tile_pool(name="state", bufs=1) as spool,
            tc.tile_pool(name="step", bufs=4) as pool,
            tc.tile_pool(name="gath", bufs=18) as gpool,
            tc.tile_pool(name="psumG", bufs=4, space="PSUM") as ppool,
            tc.tile_pool(name="psumT", bufs=2, space="PSUM") as ptpool,
        ):
            eT_sb = cpool.tile([K, n_tok2], BF16)
            if gather:
                idx_sb = cpool.tile([128, n_chunks], mybir.dt.int32)
                nc.sync.dma_start(out=idx_sb[:], in_=idx_d[:])
                ident = cpool.tile([128, 128], BF16)
                make_identity(nc, ident[:])
            else:
                nc.sync.dma_start(out=eT_sb[:], in_=eT_d[:])
            w_sb = cpool.tile([K, 512], BF16)
            nc.sync.dma_start(out=w_sb[:], in_=w_d[:])
            u_sb = cpool.tile([128, 512], BF16)
            nc.sync.dma_start(out=u_sb[:], in_=u_d[:])

            C = spool.tile([128, bc], F32)  # cell state, fwd on 0:64, bwd on 64:128
            Hn = spool.tile([128, bc], BF16)  # hidden state (matmul rhs)

            R_tiles = {}

            def trigger_chunk(c, throttle=True):
                """Kick the indirect row-gather DMA for chunk c (Pool SWDGE).
                The offset column is staged through a DVE copy so the trigger
                executes in step order: without this the Pool drains all
                queued triggers back-to-back and the resulting transpose/copy
                flood slows every scan step until the gather finishes."""
                R = gpool.tile([128, K], BF16, tag="R")
                R_tiles[c] = R
                if throttle:
                    stg = gpool.tile([128, 1], mybir.dt.int32, tag="stg")
                    nc.vector.tensor_copy(out=stg[:], in_=idx_sb[:, c : c + 1])
                    off = stg[:, 0:1]
                else:
                    off = idx_sb[:, c : c + 1]
                nc.gpsimd.indirect_dma_start(
                    out=R[:],
                    out_offset=None,
                    in_=emb_d[:],
                    in_offset=bass.IndirectOffsetOnAxis(ap=off, axis=0),
                )

            def xform_chunk(c):
                """Transpose gathered rows and cast-copy into eT (bf16)."""
                R = R_tiles.pop(c)
                pT = ptpool.tile([K, 1024], BF16, tag="pT")
                nc.tensor.transpose(out=pT[:, 0:128], in_=R[:], identity=ident[:])
                # GPSIMD cannot read PSUM: split the cast-copies between DVE
                # and ACT (_COPY_MOD controls the ratio)
                if c % _COPY_MOD[0] != _COPY_MOD[0] - 1:
                    nc.vector.tensor_copy(
                        out=eT_sb[0:K, c * 128 : (c + 1) * 128], in_=pT[:, 0:128]
                    )
                else:
                    nc.scalar.copy(
                        out=eT_sb[0:K, c * 128 : (c + 1) * 128], in_=pT[:, 0:128]
                    )

            G_tiles = {}

            def emit_e(t, first_step):
                """e-projections for step t into a fresh PSUM gate tile.
                Column layout [f|i|o|g] x bc per dir on partition halves."""
                G = ppool.tile([128, 512], F32, tag="G")
                G_tiles[t] = G
                for d in range(2):
                    ecol = eT_sb[:, t * 2 * bc + d * bc : t * 2 * bc + (d + 1) * bc]
                    for q in range(4):
                        nc.tensor.matmul(
                            G[d * 64 : (d + 1) * 64, q * bc : (q + 1) * bc],
                            lhsT=w_sb[:, d * 256 + q * 64 : d * 256 + (q + 1) * 64],
                            rhs=ecol,
                            start=(q == 0),
                            stop=(first_step and d == 1 and q == 3),
                        )

            def step(t, first_step):
                G = G_tiles.pop(t)
                if not first_step:
                    for q in range(4):
                        nc.tensor.matmul(
                            G[:, q * bc : (q + 1) * bc],
                            lhsT=u_sb[:, q * 128 : (q + 1) * 128],
                            rhs=Hn[:],
                            start=False,
                            stop=(q == 3),
                        )
                # bf16 X: 2-byte mode halves the per-column DVE cost of the
                # gate ops; c stays fp32 (sigma outputs are bounded, the
                # rounding is ~0.4% and contractive through the gates)
                X = pool.tile([128, 4 * bc], BF16, tag="X")  # [sf|si|so|tg]
                if scheme == "o1":
                    nc.scalar.activation(X[:, 0 : 3 * bc], G[:, 0 : 3 * bc], AF.Sigmoid)
                    nc.scalar.activation(
                        X[:, 3 * bc : 4 * bc], G[:, 3 * bc : 4 * bc], AF.Tanh
                    )
                elif scheme == "o3":
                    nc.scalar.activation(X[:, 0 : 2 * bc], G[:, 0 : 2 * bc], AF.Sigmoid)
                    nc.scalar.activation(
                        X[:, 3 * bc : 4 * bc], G[:, 3 * bc : 4 * bc], AF.Tanh
                    )
                    nc.scalar.activation(
                        X[:, 2 * bc : 3 * bc], G[:, 2 * bc : 3 * bc], AF.Sigmoid
                    )
                elif scheme == "o5":
                    nc.scalar.activation(X[:, 0 : 4 * bc], G[:, 0 : 4 * bc], AF.Sigmoid)
                    nc.vector.tensor_scalar(
                        out=X[:, 3 * bc : 4 * bc], in0=X[:, 3 * bc : 4 * bc],
                        scalar1=2.0, scalar2=-1.0, op0=ALU.mult, op1=ALU.add,
                    )
                elif scheme == "o6":
                    # blocks [f|i|g2|o]: on-path sigma covers f,i,g2 only;
                    # sigma(o) runs later in the ACT gap before tanh_c
                    nc.scalar.activation(X[:, 0 : 3 * bc], G[:, 0 : 3 * bc], AF.Sigmoid)
                    nc.vector.tensor_scalar(
                        out=X[:, 2 * bc : 3 * bc], in0=X[:, 2 * bc : 3 * bc],
                        scalar1=2.0, scalar2=-1.0, op0=ALU.mult, op1=ALU.add,
                    )
                    nc.scalar.activation(
                        X[:, 3 * bc : 4 * bc], G[:, 3 * bc : 4 * bc], AF.Sigmoid
                    )
                else:
                    raise ValueError(scheme)
                tg_blk = 2 if scheme == "o6" else 3
                PD = pool.tile([128, 2 * bc], BF16, tag="PD")
                nc.vector.tensor_tensor(  # sf * c
                    out=PD[:, 0:bc], in0=X[:, 0:bc], in1=C[:], op=ALU.mult
                )
                nc.vector.tensor_tensor(  # si * tg
                    out=PD[:, bc : 2 * bc], in0=X[:, bc : 2 * bc],
                    in1=X[:, tg_blk * bc : (tg_blk + 1) * bc], op=ALU.mult,
                )
                nc.vector.tensor_tensor(  # c' (in place)
                    out=C[:], in0=PD[:, 0:bc], in1=PD[:, bc : 2 * bc], op=ALU.add
                )
                TC = pool.tile([128, bc], F32, tag="TC")
                nc.scalar.activation(TC[:], C[:], AF.Tanh)
                so_blk = 3 if scheme == "o6" else 2
                nc.vector.tensor_tensor(  # h = so * tanh(c'), bf16 for next matmul
                    out=Hn[:], in0=X[:, so_blk * bc : (so_blk + 1) * bc], in1=TC[:],
                    op=ALU.mult,
                )
                return X

            for _rep in range(repeats):
                do_gather = gather and (_rep == 0 or gather_reps)
                if _rep > 0 and gather_reps:
                    tc.tile_update_base_wait()
                nc.vector.memset(C[:], 0.0)
                if do_gather:
                    for c in range(min(PFCH, n_chunks)):
                        trigger_chunk(c)
                    for c in range(min(CPF, n_chunks)):
                        xform_chunk(c)
                for tt in range(min(PF, s_len)):
                    emit_e(tt, tt == 0)
                X_last = None
                for t in range(s_len):
                    X_last = step(t, t == 0)
                    if do_gather and t % 2 == 0:
                        base = _PRO_EST_NS[0] + t * _STEP_EST_NS[0]
                        c = t // 2 + PFCH
                        if c < n_chunks:
                            with tc.tile_wait_until((base + _TRIG_OFF_NS[0]) / 1e6):
                                trigger_chunk(c)
                        c2 = t // 2 + CPF
                        if c2 < n_chunks:
                            with tc.tile_wait_until((base + _XF_OFF_NS[0]) / 1e6):
                                xform_chunk(c2)
                    if t + PF < s_len:
                        emit_e(t + PF, False)

            nc.sync.dma_start(out=y[:, 0:bc], in_=C[:])
            so_blk = 3 if scheme == "o6" else 2
            so_f32 = spool.tile([128, bc], F32)
            nc.vector.tensor_copy(
                out=so_f32[:], in_=X_last[:, so_blk * bc : (so_blk + 1) * bc]
            )
            nc.sync.dma_start(out=y[:, bc : 2 * bc], in_=so_f32[:])

    nc.compile()
    _built[key] = nc
    return nc


# reference gate row order is i,f,g,o. Column block order: f,i,o,g for
# o1/o3/o5 (sigma needs f,i,o contiguous); f,i,g,o for o6 (sigma covers f,i,g2).
_SRC_FIOG = [1, 0, 3, 2]
_SRC_FIGO = [1, 0, 2, 3]


def _prepare_in_maps(inputs, s_len=S, bc=BC, gather=True, scheme="o6"):
    x = np.asarray(inputs["x"])
    emb = np.asarray(inputs["emb"], dtype=np.float32)
    emb_aug = np.concatenate(
        [emb, np.ones((V, 1), np.float32)], axis=1
    ).astype(NP_BF16)  # ones col -> bias row of eT after transpose

    w_all = np.zeros((K, 512), np.float32)
    ubd = np.zeros((128, 512), np.float32)
    for d, sfx in enumerate("fb"):
        W_ih = np.asarray(inputs[f"W_ih_{sfx}"], np.float32)
        W_hh = np.asarray(inputs[f"W_hh_{sfx}"], np.float32)
        b = (
            np.asarray(inputs[f"b_ih_{sfx}"], np.float32)
            + np.asarray(inputs[f"b_hh_{sfx}"], np.float32)
        )
        gate_order = _SRC_FIGO if scheme == "o6" else _SRC_FIOG
        for qi, src in enumerate(gate_order):
            rows = slice(src * HID, (src + 1) * HID)
            w_all[0:E, d * 256 + qi * 64 : d * 256 + (qi + 1) * 64] = W_ih[rows].T
            w_all[E, d * 256 + qi * 64 : d * 256 + (qi + 1) * 64] = b[rows]
            ubd[d * 64 : (d + 1) * 64,
                qi * 128 + d * 64 : qi * 128 + (d + 1) * 64] = W_hh[rows].T
    if scheme in ("o5", "o6"):  # tanh(g) = 2*sigmoid(2g) - 1: double g preacts
        gq = 2 if scheme == "o6" else 3
        for d in range(2):
            w_all[:, d * 256 + gq * 64 : d * 256 + (gq + 1) * 64] *= 2.0
        ubd[:, gq * 128 : (gq + 1) * 128] *= 2.0
    w_all = w_all.astype(NP_BF16)
    ubd = ubd.astype(NP_BF16)

    n_tok2 = s_len * 2 * bc
    in_maps = []
    for core in range(N_CORES):
        rows = x[core * bc : (core + 1) * bc]  # [bc, S]
        # eT col j = t*2bc + dir*bc + b -> token x[b, t] (fwd) / x[b, S-1-t] (bwd)
        tok = np.empty((s_len, 2, bc), np.int32)
        tok[:, 0, :] = rows[:, :s_len].T
        tok[:, 1, :] = rows[:, ::-1][:, :s_len].T
        tok = tok.reshape(-1)
        m = {"w_all": w_all, "ubd": ubd}
        if gather:
            m["idx"] = np.ascontiguousarray(tok.reshape(-1, 128).T)
            m["emb"] = emb_aug
        else:
            e = emb[tok].astype(NP_BF16)  # [n_tok2, E]
            eT = np.concatenate(
                [np.ascontiguousarray(e.T), np.ones((1, n_tok2), NP_BF16)], axis=0
            )
            m["eT"] = np.ascontiguousarray(eT)
        in_maps.append(m)
    return in_maps


def _postprocess(results, inputs, bc=BC):
    fc_w = np.asarray(inputs["fc_w"], dtype=np.float32)
    fc_b = np.asarray(inputs["fc_b"], dtype=np.float32)
    h_f = np.empty((B, HID), np.float32)
    h_b = np.empty((B, HID), np.float32)
    for core in range(N_CORES):
        yv = results[core]["y"]  # [128, 2*bc]
        c_part, so_part = yv[:, 0:bc], yv[:, bc : 2 * bc]
        sl = slice(core * bc, (core + 1) * bc)
        h_f[sl] = (so_part[0:64] * np.tanh(c_part[0:64])).T
        h_b[sl] = (so_part[64:128] * np.tanh(c_part[64:128])).T
    h_cat = np.concatenate([h_f, h_b], axis=1)
    out = 1.0 / (1.0 + np.exp(-(h_cat @ fc_w.T + fc_b)))
    return out.astype(np.float32)


def kernel(x, emb, W_ih_f, W_hh_f, b_ih_f, b_hh_f, W_ih_b, W_hh_b, b_ih_b, b_hh_b,
           fc_w, fc_b, s_len=S, bc=BC, gather=True, scheme="o6"):
    inputs = dict(
        x=x, emb=emb, W_ih_f=W_ih_f, W_hh_f=W_hh_f, b_ih_f=b_ih_f, b_hh_f=b_hh_f,
        W_ih_b=W_ih_b, W_hh_b=W_hh_b, b_ih_b=b_ih_b, b_hh_b=b_hh_b,
        fc_w=fc_w, fc_b=fc_b,
    )
    nc = _build(s_len, bc, gather=gather, scheme=scheme)
    in_maps = _prepare_in_maps(inputs, s_len, bc, gather=gather, scheme=scheme)
    res = run_bass_kernel_spmd(nc, in_maps, list(range(N_CORES)))
    return _postprocess(res.results, inputs, bc)
